# revision 1
# baseline (speedup 1.0000x reference)
"""Conv4d (kernel 3^4, circular, grouped-over-time) on 8 TRN2 NeuronCores.

Math: res[b,co,t] = sum_g conv3d_valid(pad_wrap1(x[b,:,s=t-1+g]), W[g]) + bias,
with s circular over the 16 time slices.

Device scheme (per core = one (batch, 8-time-slice) shard):
  - outputs processed in pairs (t, t+1); PSUM partitions = (t-sel u, c_out)
  - contraction K = (input-slice-sel j, c_in) over pair-tiles of two
    consecutive padded slices stacked on partitions
  - per (kd,kh,kw) tap and output pair: 2 matmuls, K=128 M=128 N=512 fp32r:
      L-block: slices (t-1, t),  g = j - u      (g=-1 entry zeroed)
      H-block: slices (t+1, t+2), g = j - u + 2 (g=3 entry zeroed)
  - rhs = 3-level-AP window into the padded 18^3 slice cube, 2 output
    d-planes per matmul -> 8 PSUM banks cover the 16^3 spatial output
  - bias added during PSUM->SBUF evacuation (DVE tensor_scalar_add)
"""
import numpy as np

B, C, S, KW = 4, 64, 16, 3
SP = S + 2          # padded spatial extent
CUBE = SP * SP * SP  # 5832 padded elements per channel
NCORES = 8
TSH = S * B // NCORES  # 8 output time slices per core

_PROGRAM = None


def _build_program():
    import concourse.bacc as bacc
    import concourse.mybir as mybir
    import concourse.tile as tile

    nc = bacc.Bacc("TRN2", target_bir_lowering=False, debug=False,
                   num_devices=NCORES)
    f32r = mybir.dt.float32r
    f32 = mybir.dt.float32

    xs_d = nc.dram_tensor("xs", [5, 128, CUBE], f32r, kind="ExternalInput").ap()
    wl_d = nc.dram_tensor("wl", [128, 27 * 128], f32r, kind="ExternalInput").ap()
    wh_d = nc.dram_tensor("wh", [128, 27 * 128], f32r, kind="ExternalInput").ap()
    bias_d = nc.dram_tensor("bias2", [128, 1], f32, kind="ExternalInput").ap()
    y_d = nc.dram_tensor("y", [TSH, C, S * S * S], f32, kind="ExternalOutput").ap()

    with tile.TileContext(nc) as tc:
        with (
            tc.tile_pool(name="xp", bufs=1) as xpool,
            tc.tile_pool(name="wp", bufs=1) as wpool,
            tc.tile_pool(name="st", bufs=2) as spool,
            tc.tile_pool(name="ps", bufs=8, space="PSUM") as pspool,
        ):
            # Issue order matters: the first matmuls (pair 0, chunk 0) need
            # only the first tap-group of weights and the first plane-group
            # of xt0/xt1.  Interleave small pieces, weights on the gpsimd
            # DMA queue and x on the sync queue so they stream in parallel.
            wlt = wpool.tile([128, 27 * 128], f32r)
            wht = wpool.tile([128, 27 * 128], f32r)
            bias_t = wpool.tile([128, 1], f32)
            xts = []
            for k in range(5):
                xt = xpool.tile([128, CUBE], f32r, name=f"xt{k}")
                xts.append(xt)
            piece = 6 * SP * SP  # 6 d-planes
            wpiece = 7 * 128     # 7 taps of weights

            def wdma(p):
                lo, hi = p * wpiece, min((p + 1) * wpiece, 27 * 128)
                nc.gpsimd.dma_start(wlt[:, lo:hi], wl_d[:, lo:hi])
                nc.gpsimd.dma_start(wht[:, lo:hi], wh_d[:, lo:hi])

            def xdma(k, p):
                nc.sync.dma_start(
                    xts[k][:, p * piece:(p + 1) * piece],
                    xs_d[k][:, p * piece:(p + 1) * piece],
                )

            wdma(0)
            xdma(0, 0)
            xdma(1, 0)
            nc.gpsimd.dma_start(bias_t[:], bias_d)
            wdma(1)
            xdma(0, 1)
            xdma(1, 1)
            wdma(2)
            xdma(0, 2)
            xdma(1, 2)
            wdma(3)
            for k in (2, 3, 4):
                for p in range(3):
                    xdma(k, p)

            xvs = [xt.rearrange("p (d h w) -> p d h w", d=SP, h=SP, w=SP)
                   for xt in xts]

            for u in range(TSH // 2):  # output pair
                stage = spool.tile([128, S * S * S], f32, name="stage")
                for c in range(8):  # 2 output d-planes per chunk
                    bank = pspool.tile([128, 512], f32, name="bank")
                    nmm = 0
                    for kd in range(KW):
                        for kh in range(KW):
                            for kw in range(KW):
                                i = (kd * KW + kh) * KW + kw
                                for wt, xv in ((wlt, xvs[u]), (wht, xvs[u + 1])):
                                    rhs = xv[:, 2 * c + kd:2 * c + kd + 2,
                                             kh:kh + S, kw:kw + S]
                                    nc.tensor.matmul(
                                        bank[:],
                                        wt[:, i * 128:(i + 1) * 128],
                                        rhs,
                                        start=(nmm == 0), stop=(nmm == 53),
                                    )
                                    nmm += 1
                    nc.vector.tensor_scalar_add(
                        stage[:, c * 512:(c + 1) * 512], bank[:], bias_t[:]
                    )
                    nc.sync.dma_start(
                        y_d[2 * u][:, c * 512:(c + 1) * 512],
                        stage[0:C, c * 512:(c + 1) * 512],
                    )
                    nc.sync.dma_start(
                        y_d[2 * u + 1][:, c * 512:(c + 1) * 512],
                        stage[C:128, c * 512:(c + 1) * 512],
                    )

    nc.compile()
    return nc


def _host_prep(x, weight, bias):
    """Build per-core input maps."""
    # padded slices: xp[b, s] = wrap-pad1 of x[b,:,s] -> (C, 18,18,18)
    xpad = np.pad(x, ((0, 0), (0, 0), (0, 0), (1, 1), (1, 1), (1, 1)),
                  mode="wrap").astype(np.float32)  # (B, C, S, 18,18,18)

    # weight block-banded lhsT tiles: [128=(j,ci), 27*128=(tap,(u,co))]
    wl = np.zeros((128, 27, 128), dtype=np.float32)
    wh = np.zeros((128, 27, 128), dtype=np.float32)
    for kd in range(KW):
        for kh in range(KW):
            for kw in range(KW):
                i = (kd * KW + kh) * KW + kw
                for j in range(2):
                    for u in range(2):
                        gl = j - u
                        if 0 <= gl < KW:
                            wl[j * C:(j + 1) * C, i, u * C:(u + 1) * C] = \
                                weight[gl, :, :, kd, kh, kw].T
                        gh = j - u + 2
                        if 0 <= gh < KW:
                            wh[j * C:(j + 1) * C, i, u * C:(u + 1) * C] = \
                                weight[gh, :, :, kd, kh, kw].T
    wl = wl.reshape(128, 27 * 128)
    wh = wh.reshape(128, 27 * 128)
    bias2 = np.concatenate([bias, bias]).astype(np.float32).reshape(128, 1)

    in_maps = []
    for core in range(NCORES):
        b = core // 2
        t0 = TSH * (core % 2)
        xs = np.empty((5, 128, CUBE), dtype=np.float32)
        for k in range(5):
            sa = (t0 - 1 + 2 * k) % S
            sb = (t0 + 2 * k) % S
            xs[k, 0:C] = xpad[b, :, sa].reshape(C, CUBE)
            xs[k, C:128] = xpad[b, :, sb].reshape(C, CUBE)
        in_maps.append({"xs": xs, "wl": wl, "wh": wh, "bias2": bias2})
    return in_maps


LAST_RESULTS = None


def kernel(x, weight, bias, _trace=False):
    global _PROGRAM, LAST_RESULTS
    from concourse import bass_utils

    x = np.asarray(x, dtype=np.float32)
    weight = np.asarray(weight, dtype=np.float32)
    bias = np.asarray(bias, dtype=np.float32)

    if _PROGRAM is None:
        _PROGRAM = _build_program()
    nc = _PROGRAM

    in_maps = _host_prep(x, weight, bias)
    res = bass_utils.run_bass_kernel_spmd(
        nc, in_maps, core_ids=list(range(NCORES)), trace=_trace
    )
    LAST_RESULTS = res

    out = np.empty((B, C, S, S, S, S), dtype=np.float32)
    for core in range(NCORES):
        b = core // 2
        t0 = TSH * (core % 2)
        y = res.results[core]["y"]  # (TSH, C, 4096)
        out[b, :, t0:t0 + TSH] = y.transpose(1, 0, 2).reshape(C, TSH, S, S, S)
    return out



# revision 2
# speedup vs baseline: 1.1732x; 1.1732x over previous
"""Conv4d (kernel 3^4, circular, grouped-over-time) on 8 TRN2 NeuronCores.

Math: res[b,co,t] = sum_g conv3d_valid(pad_wrap1(x[b,:,s=t-1+g]), W[g]) + bias,
with s circular over the 16 time slices.

Device scheme (per core = one (batch, 8-time-slice) shard):
  - outputs processed in pairs (t, t+1); PSUM partitions = (t-sel u, c_out)
  - contraction K = (input-slice-sel j, c_in) over pair-tiles of two
    consecutive padded slices stacked on partitions
  - per (kd,kh,kw) tap and output pair: 2 matmuls, K=128 M=128 N=512 bf16:
      L-block: slices (t-1, t),  g = j - u      (g=-1 entry zeroed)
      H-block: slices (t+1, t+2), g = j - u + 2 (g=3 entry zeroed)
  - loop order tap-outer / chunk-inner: 8 consecutive matmuls share one
    lhsT slice, minimizing unhidden LDWEIGHTS time on the PE
  - rhs = 3-level-AP window into the padded 18^3 slice cube, 2 output
    d-planes per matmul -> 8 PSUM banks cover the 16^3 spatial output
  - bias added during PSUM->SBUF evacuation (DVE tensor_scalar_add)
"""
import numpy as np

B, C, S, KW = 4, 64, 16, 3
SP = S + 2          # padded spatial extent
CUBE = SP * SP * SP  # 5832 padded elements per channel
NCORES = 8
TSH = S * B // NCORES  # 8 output time slices per core

_PROGRAM = None


def _build_program():
    import concourse.bacc as bacc
    import concourse.mybir as mybir
    import concourse.tile as tile

    nc = bacc.Bacc("TRN2", target_bir_lowering=False, debug=False,
                   num_devices=NCORES)
    bf16 = mybir.dt.bfloat16
    f32 = mybir.dt.float32

    xs_d = nc.dram_tensor("xs", [5, 128, CUBE], bf16, kind="ExternalInput").ap()
    wl_d = nc.dram_tensor("wl", [128, 27 * 128], bf16, kind="ExternalInput").ap()
    wh_d = nc.dram_tensor("wh", [128, 27 * 128], bf16, kind="ExternalInput").ap()
    bias_d = nc.dram_tensor("bias2", [128, 1], f32, kind="ExternalInput").ap()
    y_d = nc.dram_tensor("y", [TSH, C, S * S * S], f32, kind="ExternalOutput").ap()

    with tile.TileContext(nc) as tc:
        with (
            tc.tile_pool(name="xp", bufs=1) as xpool,
            tc.tile_pool(name="wp", bufs=1) as wpool,
            tc.tile_pool(name="st", bufs=2) as spool,
            tc.tile_pool(name="ps", bufs=8, space="PSUM") as pspool,
        ):
            wlt = wpool.tile([128, 27 * 128], bf16)
            wht = wpool.tile([128, 27 * 128], bf16)
            bias_t = wpool.tile([128, 1], f32)
            xts = [xpool.tile([128, CUBE], bf16, name=f"xt{k}") for k in range(5)]

            # DMA schedule: the first matmuls (pair 0, L-block, tap 0) touch
            # ALL d-planes of cube 0 within the first 8 matmuls, plus the
            # first weight tap.  Load the first wl piece and cube 0 up front
            # on separate queues, then stream the rest.
            half = 9 * SP * SP  # 9 d-planes per piece

            def xdma(q, k, p):
                q.dma_start(xts[k][:, p * half:(p + 1) * half],
                            xs_d[k][:, p * half:(p + 1) * half])

            wpiece = 7 * 128

            def wdma(q, t, p):
                lo, hi = p * wpiece, min((p + 1) * wpiece, 27 * 128)
                q.dma_start(t[:, lo:hi], (wl_d if t is wlt else wh_d)[:, lo:hi])

            wdma(nc.gpsimd, wlt, 0)          # taps 0-6 of L weights
            xdma(nc.sync, 0, 0)              # cube 0 planes 0-8
            xdma(nc.sync, 0, 1)              # cube 0 planes 9-17
            nc.gpsimd.dma_start(bias_t[:], bias_d)
            for p in (1, 2, 3):
                wdma(nc.gpsimd, wlt, p)
            xdma(nc.sync, 1, 0)
            xdma(nc.sync, 1, 1)
            for p in range(4):
                wdma(nc.gpsimd, wht, p)
            for k in (2, 3, 4):
                xdma(nc.sync, k, 0)
                xdma(nc.sync, k, 1)

            xvs = [xt.rearrange("p (d h w) -> p d h w", d=SP, h=SP, w=SP)
                   for xt in xts]

            for u in range(TSH // 2):  # output pair
                banks = [pspool.tile([128, 512], f32, name="bank")
                         for _ in range(8)]
                stage = spool.tile([128, S * S * S], f32, name="stage")
                for blk in range(2):   # L (wlt) then H (wht)
                    wt = wlt if blk == 0 else wht
                    xv = xvs[u + blk]
                    for kd in range(KW):
                        for kh in range(KW):
                            for kw in range(KW):
                                i = (kd * KW + kh) * KW + kw
                                lhsT = wt[:, i * 128:(i + 1) * 128]
                                for c in range(8):
                                    rhs = xv[:, 2 * c + kd:2 * c + kd + 2,
                                             kh:kh + S, kw:kw + S]
                                    nc.tensor.matmul(
                                        banks[c][:], lhsT, rhs,
                                        start=(blk == 0 and i == 0),
                                        stop=(blk == 1 and i == 26),
                                    )
                for c in range(8):
                    nc.vector.tensor_scalar_add(
                        stage[:, c * 512:(c + 1) * 512], banks[c][:], bias_t[:]
                    )
                    nc.sync.dma_start(
                        y_d[2 * u][:, c * 512:(c + 1) * 512],
                        stage[0:C, c * 512:(c + 1) * 512],
                    )
                    nc.sync.dma_start(
                        y_d[2 * u + 1][:, c * 512:(c + 1) * 512],
                        stage[C:128, c * 512:(c + 1) * 512],
                    )

    nc.compile()
    return nc


def _host_prep(x, weight, bias):
    """Build per-core input maps (bf16 activations/weights, f32 bias)."""
    import ml_dtypes

    xpad = np.pad(x, ((0, 0), (0, 0), (0, 0), (1, 1), (1, 1), (1, 1)),
                  mode="wrap").astype(ml_dtypes.bfloat16)  # (B, C, S, 18,18,18)

    # weight block-banded lhsT tiles: [128=(j,ci), 27*128=(tap,(u,co))]
    wl = np.zeros((128, 27, 128), dtype=np.float32)
    wh = np.zeros((128, 27, 128), dtype=np.float32)
    for kd in range(KW):
        for kh in range(KW):
            for kw in range(KW):
                i = (kd * KW + kh) * KW + kw
                for j in range(2):
                    for u in range(2):
                        gl = j - u
                        if 0 <= gl < KW:
                            wl[j * C:(j + 1) * C, i, u * C:(u + 1) * C] = \
                                weight[gl, :, :, kd, kh, kw].T
                        gh = j - u + 2
                        if 0 <= gh < KW:
                            wh[j * C:(j + 1) * C, i, u * C:(u + 1) * C] = \
                                weight[gh, :, :, kd, kh, kw].T
    wl = wl.reshape(128, 27 * 128).astype(ml_dtypes.bfloat16)
    wh = wh.reshape(128, 27 * 128).astype(ml_dtypes.bfloat16)
    bias2 = np.concatenate([bias, bias]).astype(np.float32).reshape(128, 1)

    in_maps = []
    for core in range(NCORES):
        b = core // 2
        t0 = TSH * (core % 2)
        xs = np.empty((5, 128, CUBE), dtype=ml_dtypes.bfloat16)
        for k in range(5):
            sa = (t0 - 1 + 2 * k) % S
            sb = (t0 + 2 * k) % S
            xs[k, 0:C] = xpad[b, :, sa].reshape(C, CUBE)
            xs[k, C:128] = xpad[b, :, sb].reshape(C, CUBE)
        in_maps.append({"xs": xs, "wl": wl, "wh": wh, "bias2": bias2})
    return in_maps


LAST_RESULTS = None


def kernel(x, weight, bias, _trace=False):
    global _PROGRAM, LAST_RESULTS
    from concourse import bass_utils

    x = np.asarray(x, dtype=np.float32)
    weight = np.asarray(weight, dtype=np.float32)
    bias = np.asarray(bias, dtype=np.float32)

    if _PROGRAM is None:
        _PROGRAM = _build_program()
    nc = _PROGRAM

    in_maps = _host_prep(x, weight, bias)
    res = bass_utils.run_bass_kernel_spmd(
        nc, in_maps, core_ids=list(range(NCORES)), trace=_trace
    )
    LAST_RESULTS = res

    out = np.empty((B, C, S, S, S, S), dtype=np.float32)
    for core in range(NCORES):
        b = core // 2
        t0 = TSH * (core % 2)
        y = res.results[core]["y"]  # (TSH, C, 4096)
        out[b, :, t0:t0 + TSH] = y.transpose(1, 0, 2).reshape(C, TSH, S, S, S)
    return out


# revision 5
# speedup vs baseline: 1.2132x; 1.0341x over previous
"""Conv4d (kernel 3^4, circular, grouped-over-time) on 8 TRN2 NeuronCores.

Math: res[b,co,t] = sum_g conv3d_valid(pad_wrap1(x[b,:,s=t-1+g]), W[g]) + bias,
with s circular over the 16 time slices.

Device scheme (per core = one (batch, 8-time-slice) shard):
  - outputs processed in pairs (t, t+1); PSUM partitions = (t-sel u, c_out)
  - contraction K = (input-slice-sel j, c_in) over pair-tiles of two
    consecutive padded slices stacked on partitions
  - per (kd,kh,kw) tap and output pair: 2 matmuls, K=128 M=128 N=512 bf16:
      L-block: slices (t-1, t),  g = j - u      (g=-1 entry zeroed)
      H-block: slices (t+1, t+2), g = j - u + 2 (g=3 entry zeroed)
  - x staged in SBUF as THREE kw-pre-shifted 16-wide cube copies so every
    rhs window is a contiguous 32B-aligned 2-level AP (a 2-byte-misaligned
    bf16 moving window costs ~35ns/matmul on the PE)
  - loop order tap-outer / chunk-inner: 8 consecutive matmuls share one
    lhsT slice, minimizing unhidden LDWEIGHTS time; taps ordered kw-major
    (0, 2, 1) to relax DMA deadlines of the shifted copies
  - PSUM evacuation split across DVE (tensor_scalar_add) and Act
    (activation Identity + bias) into a bf16 stage; host upcasts to f32
"""
import numpy as np

B, C, S, KW = 4, 64, 16, 3
SP = S + 2            # padded d/h extent
CUBE = SP * SP * S    # 5184: one 16-wide shifted cube copy per channel
NCORES = 8
TSH = S * B // NCORES  # 8 output time slices per core
KWORD = (0, 2, 1)      # kw processing order (shifted-copy DMA slack)

_PROGRAM = None


def _build_program():
    import concourse.bacc as bacc
    import concourse.mybir as mybir
    import concourse.tile as tile

    nc = bacc.Bacc("TRN2", target_bir_lowering=False, debug=False,
                   num_devices=NCORES)
    bf16 = mybir.dt.bfloat16
    f32 = mybir.dt.float32
    IDENT = mybir.ActivationFunctionType.Identity

    xs_d = [nc.dram_tensor(f"xs{kw}", [5, 128, CUBE], bf16,
                           kind="ExternalInput").ap() for kw in range(KW)]
    wl_d = nc.dram_tensor("wl", [128, 27 * 128], bf16, kind="ExternalInput").ap()
    wh_d = nc.dram_tensor("wh", [128, 27 * 128], bf16, kind="ExternalInput").ap()
    bias_d = nc.dram_tensor("bias2", [128, 1], f32, kind="ExternalInput").ap()
    y_d = nc.dram_tensor("y", [TSH, C, S * S * S], bf16,
                         kind="ExternalOutput").ap()

    with tile.TileContext(nc) as tc:
        with (
            tc.tile_pool(name="xp", bufs=1) as xpool,
            tc.tile_pool(name="wp", bufs=1) as wpool,
            tc.tile_pool(name="st", bufs=2) as spool,
            tc.tile_pool(name="ps", bufs=8, space="PSUM") as pspool,
        ):
            wlt = wpool.tile([128, 27 * 128], bf16)
            wht = wpool.tile([128, 27 * 128], bf16)
            bias_t = wpool.tile([128, 1], f32)
            xts = [[xpool.tile([128, CUBE], bf16, name=f"xt{kw}_{k}")
                    for k in range(5)] for kw in range(KW)]

            half = 9 * SP * S  # 9 d-planes

            def xdma(q, kw, k, p):
                q.dma_start(xts[kw][k][:, p * half:(p + 1) * half],
                            xs_d[kw][k][:, p * half:(p + 1) * half])

            wpiece = 7 * 128

            def wdma(q, t, p):
                lo, hi = p * wpiece, min((p + 1) * wpiece, 27 * 128)
                q.dma_start(t[:, lo:hi], (wl_d if t is wlt else wh_d)[:, lo:hi])

            # Early: first matmuls (kw=0, tap 0, chunks 0-7) touch all of
            # C0-cube0 plus the first wl piece.  Weights lead on gpsimd;
            # C2/C1 copies follow on gpsimd/scalar (needed ~25us/~45us in).
            wdma(nc.gpsimd, wlt, 0)
            xdma(nc.sync, 0, 0, 0)
            xdma(nc.sync, 0, 0, 1)
            xdma(nc.gpsimd, 2, 0, 0)
            xdma(nc.gpsimd, 2, 0, 1)
            nc.gpsimd.dma_start(bias_t[:], bias_d)
            xdma(nc.sync, 0, 1, 0)
            xdma(nc.sync, 0, 1, 1)
            for p in (1, 2, 3):
                wdma(nc.gpsimd, wlt, p)
            xdma(nc.scalar, 1, 0, 0)
            xdma(nc.scalar, 1, 0, 1)
            for p in range(4):
                wdma(nc.gpsimd, wht, p)
            for k in (1, 2, 3, 4):
                xdma(nc.gpsimd, 2, k, 0)
                xdma(nc.gpsimd, 2, k, 1)
            xdma(nc.scalar, 1, 1, 0)
            xdma(nc.scalar, 1, 1, 1)
            for k in (2, 3, 4):
                xdma(nc.sync, 0, k, 0)
                xdma(nc.sync, 0, k, 1)
                xdma(nc.scalar, 1, k, 0)
                xdma(nc.scalar, 1, k, 1)

            # [p, d(18), hw(18 h-rows of 16)]
            xvs = [[xt.rearrange("p (d hw) -> p d hw", d=SP, hw=SP * S)
                    for xt in row] for row in xts]

            for u in range(TSH // 2):  # output pair
                banks = [pspool.tile([128, 512], f32, name="bank")
                         for _ in range(8)]
                stage = spool.tile([128, S * S * S], bf16, name="stage")
                slot = 0
                for blk in range(2):   # L (wlt) then H (wht)
                    wt = wlt if blk == 0 else wht
                    for kwi, kw in enumerate(KWORD):
                        xv = xvs[kw][u + blk]
                        for kd in range(KW):
                            for kh in range(KW):
                                i = kwi * 9 + kd * KW + kh
                                lhsT = wt[:, i * 128:(i + 1) * 128]
                                for c in range(8):
                                    rhs = xv[:, 2 * c + kd:2 * c + kd + 2,
                                             kh * S:(kh + S) * S]
                                    nc.tensor.matmul(
                                        banks[c][:], lhsT, rhs,
                                        start=(slot == 0), stop=(slot == 53),
                                    )
                                slot += 1
                for c in range(8):
                    sl = stage[:, c * 512:(c + 1) * 512]
                    if c % 2 == 0:
                        nc.vector.tensor_scalar_add(sl, banks[c][:], bias_t[:])
                    else:
                        nc.scalar.activation(sl, banks[c][:], IDENT,
                                             bias=bias_t[:])
                qy = nc.sync if u % 2 == 0 else nc.gpsimd
                qy.dma_start(y_d[2 * u], stage[0:C, :])
                qy.dma_start(y_d[2 * u + 1], stage[C:128, :])

    nc.compile()
    return nc


def _host_prep(x, weight, bias):
    """Build per-core input maps (bf16 activations/weights, f32 bias)."""
    import ml_dtypes

    xpad = np.pad(x, ((0, 0), (0, 0), (0, 0), (1, 1), (1, 1), (1, 1)),
                  mode="wrap").astype(np.float32)  # (B, C, S, 18,18,18)

    # kw-pre-shifted 16-wide copies: xsh[kw][b,ci,s,d,h,w16] = xpad[..., w16+kw]
    xsh = [np.ascontiguousarray(xpad[..., kw:kw + S]).astype(ml_dtypes.bfloat16)
           for kw in range(KW)]

    # weight block-banded lhsT tiles: [128=(j,ci), 27*128=(tap,(u,co))]
    # tap index i = kwi*9 + kd*3 + kh with kw = KWORD[kwi]
    wl = np.zeros((128, 27, 128), dtype=np.float32)
    wh = np.zeros((128, 27, 128), dtype=np.float32)
    for kwi, kw in enumerate(KWORD):
        for kd in range(KW):
            for kh in range(KW):
                i = kwi * 9 + kd * KW + kh
                for j in range(2):
                    for u in range(2):
                        gl = j - u
                        if 0 <= gl < KW:
                            wl[j * C:(j + 1) * C, i, u * C:(u + 1) * C] = \
                                weight[gl, :, :, kd, kh, kw].T
                        gh = j - u + 2
                        if 0 <= gh < KW:
                            wh[j * C:(j + 1) * C, i, u * C:(u + 1) * C] = \
                                weight[gh, :, :, kd, kh, kw].T
    wl = wl.reshape(128, 27 * 128).astype(ml_dtypes.bfloat16)
    wh = wh.reshape(128, 27 * 128).astype(ml_dtypes.bfloat16)
    bias2 = np.concatenate([bias, bias]).astype(np.float32).reshape(128, 1)

    in_maps = []
    for core in range(NCORES):
        b = core // 2
        t0 = TSH * (core % 2)
        m = {"wl": wl, "wh": wh, "bias2": bias2}
        for kw in range(KW):
            xs = np.empty((5, 128, CUBE), dtype=ml_dtypes.bfloat16)
            for k in range(5):
                sa = (t0 - 1 + 2 * k) % S
                sb = (t0 + 2 * k) % S
                xs[k, 0:C] = xsh[kw][b, :, sa].reshape(C, CUBE)
                xs[k, C:128] = xsh[kw][b, :, sb].reshape(C, CUBE)
            m[f"xs{kw}"] = xs
        in_maps.append(m)
    return in_maps


LAST_RESULTS = None


def kernel(x, weight, bias, _trace=False):
    global _PROGRAM, LAST_RESULTS
    from concourse import bass_utils

    x = np.asarray(x, dtype=np.float32)
    weight = np.asarray(weight, dtype=np.float32)
    bias = np.asarray(bias, dtype=np.float32)

    if _PROGRAM is None:
        _PROGRAM = _build_program()
    nc = _PROGRAM

    in_maps = _host_prep(x, weight, bias)
    res = bass_utils.run_bass_kernel_spmd(
        nc, in_maps, core_ids=list(range(NCORES)), trace=_trace
    )
    LAST_RESULTS = res

    out = np.empty((B, C, S, S, S, S), dtype=np.float32)
    for core in range(NCORES):
        b = core // 2
        t0 = TSH * (core % 2)
        y = np.asarray(res.results[core]["y"], dtype=np.float32)  # (TSH, C, 4096)
        out[b, :, t0:t0 + TSH] = y.transpose(1, 0, 2).reshape(C, TSH, S, S, S)
    return out
